# revision 1
# baseline (speedup 1.0000x reference)
"""CRF forward (log partition) on 8 NeuronCores, data-parallel over batch.

Math: the forward recurrence is run in probability space instead of log
space.  With E = exp(transitions) and G_t = exp(emissions_t), the CRF
recurrence alpha_{t+1} = logit_t + LSE_j(T + alpha_t) becomes the linear
recurrence P_{t+1} = G_t o (E @ P_t), with per-column renormalization every
few steps to stay in fp32 range (the log of each column's normalizer is
accumulated host-side from the stored reciprocals).

Variable lengths are handled by rewriting emissions: for t >= len[b] the
emission row is onehot(STOP), which makes the chain absorb into STOP (the
absorbing step computes exactly LSE_j(T[STOP,j] + alpha_j), i.e. the final
answer) and then drift by T[STOP,STOP] per extra step, which is corrected
exactly on the host: norm += (len - S) * T[STOP,STOP].

Per-core layout: 128 sequences are packed as 2 groups x 64 columns; the
state tile is [90, 64] (rows = 2 stacked copies of the 45 labels) and the
transition matmul uses blockdiag(E^T, E^T) as the stationary lhsT, so one
PE matmul advances all 128 sequences.
"""

import numpy as np

import concourse.bacc as bacc
import concourse.bass as bass
import concourse.mybir as mybir
import concourse.tile as tile
from concourse.bass_utils import run_bass_kernel_spmd

L = 45
START = 43
STOP = 44
B = 1024
S = 512
NCORES = 8
BPC = B // NCORES          # 128 sequences per core
NG = 2                     # groups per core
WCOL = BPC // NG           # 64 columns per group
PR = NG * L                # 90 partition rows for packed state
TSTEPS = S + 1             # +1 appended absorb step
RENORM = 6
NCHUNK = 9
CHUNK = TSTEPS // NCHUNK   # 57 steps per G chunk
assert CHUNK * NCHUNK == TSTEPS

F32 = mybir.dt.float32

_EVENTS = [t for t in range(TSTEPS) if (t + 1) % RENORM == 0 or t == TSTEPS - 1]
NEV = len(_EVENTS)


def _build_nc():
    # Bacc (not raw Bass): its legalization splits multi-sem waits into
    # standalone event-semaphore instructions, which walrus codegen requires.
    nc = bacc.Bacc("TRN2", target_bir_lowering=False, debug=False, num_devices=NCORES)
    g_dram = nc.dram_tensor("g", [PR, TSTEPS * WCOL], F32, kind="ExternalInput")
    e2t_dram = nc.dram_tensor("e2t", [PR, PR], F32, kind="ExternalInput")
    onesg_dram = nc.dram_tensor("onesg", [PR, NG], F32, kind="ExternalInput")
    indic_dram = nc.dram_tensor("indic", [NG, PR], F32, kind="ExternalInput")
    wout_dram = nc.dram_tensor("wout", [PR, WCOL], F32, kind="ExternalOutput")
    rstrip_dram = nc.dram_tensor("rstrip", [NG, NEV * WCOL], F32, kind="ExternalOutput")

    with tile.TileContext(nc) as tc:
        with (
            tc.tile_pool(name="const", bufs=1) as const_pool,
            tc.tile_pool(name="gchunks", bufs=NCHUNK) as g_pool,
            tc.tile_pool(name="state", bufs=3) as state_pool,
            tc.tile_pool(name="strip", bufs=1) as strip_pool,
            tc.tile_pool(name="ps_s", bufs=3, space="PSUM") as ps_s,
            tc.tile_pool(name="ps_n", bufs=2, space="PSUM") as ps_n,
            tc.tile_pool(name="ps_bc", bufs=2, space="PSUM") as ps_bc,
        ):
            # Matmult instructions encode only a few sem waits; DMAs can fan
            # out over many HWDGE queues (many sems).  Stage every matmul
            # input through a DVE copy so matmuls wait on compute sems only.
            e2t_st = const_pool.tile([PR, PR], F32, tag="e2t_st")
            nc.sync.dma_start(e2t_st[:], e2t_dram[:])
            e2t = const_pool.tile([PR, PR], F32, tag="e2t")
            nc.vector.tensor_copy(e2t[:], e2t_st[:])
            onesg_st = const_pool.tile([PR, NG], F32, tag="onesg_st")
            nc.sync.dma_start(onesg_st[:], onesg_dram[:])
            onesg = const_pool.tile([PR, NG], F32, tag="onesg")
            nc.vector.tensor_copy(onesg[:], onesg_st[:])
            indic_st = const_pool.tile([NG, PR], F32, tag="indic_st")
            nc.sync.dma_start(indic_st[:], indic_dram[:])
            indic = const_pool.tile([NG, PR], F32, tag="indic")
            nc.vector.tensor_copy(indic[:], indic_st[:])

            gtiles = []
            for c in range(NCHUNK):
                gt = g_pool.tile([PR, CHUNK * WCOL], F32, tag="g")
                nc.sync.dma_start(
                    gt[:], g_dram[:, c * CHUNK * WCOL : (c + 1) * CHUNK * WCOL]
                )
                gtiles.append(gt)

            rstrip = strip_pool.tile([NG, NEV * WCOL], F32, tag="rstrip")

            # Step 0 is folded host-side: the t=0 block of g already holds
            # W_0 = G_0 * E[:, START], the unnormalized state after step 0.
            pcur = state_pool.tile([PR, WCOL], F32, tag="w")
            nc.vector.tensor_copy(pcur[:], gtiles[0][:, 0:WCOL])

            ev = 0
            for t in range(1, TSTEPS):
                s_ps = ps_s.tile([PR, WCOL], F32, tag="s")
                nc.tensor.matmul(s_ps[:], e2t[:], pcur[:], start=True, stop=True)
                gslice = gtiles[t // CHUNK][
                    :, (t % CHUNK) * WCOL : (t % CHUNK + 1) * WCOL
                ]
                w = state_pool.tile([PR, WCOL], F32, tag="w")
                nc.vector.tensor_mul(w[:], gslice, s_ps[:])
                if (t + 1) % RENORM == 0 or t == TSTEPS - 1:
                    n_ps = ps_n.tile([NG, WCOL], F32, tag="n")
                    nc.tensor.matmul(n_ps[:], onesg[:], w[:], start=True, stop=True)
                    rslot = rstrip[:, ev * WCOL : (ev + 1) * WCOL]
                    nc.vector.reciprocal(rslot, n_ps[:])
                    bc_ps = ps_bc.tile([PR, WCOL], F32, tag="bc")
                    nc.tensor.matmul(bc_ps[:], indic[:], rslot, start=True, stop=True)
                    pnew = state_pool.tile([PR, WCOL], F32, tag="w")
                    nc.vector.tensor_mul(pnew[:], w[:], bc_ps[:])
                    pcur = pnew
                    ev += 1
                else:
                    pcur = w

            assert ev == NEV
            nc.sync.dma_start(wout_dram[:], pcur[:])
            nc.sync.dma_start(rstrip_dram[:], rstrip[:])

    nc.compile()
    return nc


_NC_CACHE = {}


def _get_nc():
    if "nc" not in _NC_CACHE:
        _NC_CACHE["nc"] = _build_nc()
    return _NC_CACHE["nc"]


def _prep_inputs(logits, lens, transitions):
    """Host-side preprocessing: exp + absorb-rewrite + per-core packing."""
    logits = np.asarray(logits, np.float32)
    lens = np.asarray(lens, np.int32)
    T = np.asarray(transitions, np.float32)

    E = np.exp(T.astype(np.float32))
    e2t = np.zeros((PR, PR), np.float32)
    e2t[:L, :L] = E.T
    e2t[L:, L:] = E.T

    onesg = np.zeros((PR, NG), np.float32)
    onesg[:L, 0] = 1.0
    onesg[L:, 1] = 1.0
    indic = np.ascontiguousarray(onesg.T)

    G = np.exp(logits)
    absorbed = np.arange(S)[None, :, None] >= lens[:, None, None]
    onehot = np.zeros(L, np.float32)
    onehot[STOP] = 1.0
    G = np.where(absorbed, onehot[None, None, :], G)
    G = np.concatenate(
        [G, np.broadcast_to(onehot, (B, 1, L)).astype(np.float32)], axis=1
    )  # [B, TSTEPS, L]

    # Fold step 0: the t=0 block becomes W_0 = G_0 * E[:, START].
    G[:, 0, :] *= E[:, START][None, :]

    in_maps = []
    for c in range(NCORES):
        gc = G[c * BPC : (c + 1) * BPC].reshape(NG, WCOL, TSTEPS, L)
        g_in = np.ascontiguousarray(
            np.transpose(gc, (0, 3, 2, 1)).reshape(PR, TSTEPS * WCOL)
        )
        in_maps.append({"g": g_in, "e2t": e2t, "onesg": onesg, "indic": indic})
    return in_maps


def _postprocess(results, lens, transitions):
    lens = np.asarray(lens, np.int64)
    T = np.asarray(transitions, np.float32)
    t_ss = np.float64(T[STOP, STOP])
    norm = np.empty(B, np.float64)
    for c in range(NCORES):
        wout = np.asarray(results[c]["wout"])      # [PR, WCOL]
        rstrip = np.asarray(results[c]["rstrip"])  # [NG, NEV*WCOL]
        rs = rstrip.reshape(NG, NEV, WCOL).astype(np.float64)
        z = -np.log(rs).sum(axis=1)                # [NG, WCOL]
        for g in range(NG):
            pstop = wout[g * L + STOP, :].astype(np.float64)
            sl = slice(c * BPC + g * WCOL, c * BPC + (g + 1) * WCOL)
            norm[sl] = np.log(pstop) + z[g] + (lens[sl] - S) * t_ss
    return norm.astype(np.float32)


def kernel(logits, lens, transitions):
    nc = _get_nc()
    in_maps = _prep_inputs(logits, lens, transitions)
    res = run_bass_kernel_spmd(nc, in_maps, list(range(NCORES)))
    return _postprocess(res.results, lens, transitions)



# revision 4
# speedup vs baseline: 4.8430x; 4.8430x over previous
"""CRF forward (log partition) on 8 NeuronCores — chunked-parallel recurrence.

Math (vs. the serial baseline): the probability-space recurrence
P_{t+1} = G_t o (E @ P_t) contracts direction exponentially fast (products of
positive matrices), so time is split into C=8 chunks of W=64 steps run as
INDEPENDENT parallel streams, each warm-started WARM=8 steps early from an
all-ones state.  Host-side stitching recovers log Z from per-chunk boundary
row-sum ratios (the warmup constant cancels in the ratio); the measured
direction error after 8 steps is ~1e-5, far below the bf16 noise floor.

Range control without on-device renorm: each active emission row is
prescaled host-side by softmax (exp(logit - LSE)) times e^{-gamma}; the exact
correction sum_t (LSE + gamma) is added back on the host.  Absorbed steps
(t >= len) park the sequence's STOP projection in a dedicated 46th row per
group whose self-transition is exactly 1.0, so parked values are bit-stable
in bf16 over hundreds of steps.

Per-core layout: 128 sequences as 2 groups x 64 columns; state tile [92, 64]
(2 stacked copies of 46 rows = 45 labels + park), lhsT = blockdiag(Ebar^T,
Ebar^T) in bf16.  Per step: one PE matmul + one elementwise mul; chunks 0-3
multiply on DVE (PSUM operand), chunks 4-7 copy PSUM->SBUF on the Scalar
engine and multiply on GPSIMD, so the three elementwise-capable engines split
the per-step work and the 8 independent chains hide inter-engine latency.
"""

import numpy as np
import ml_dtypes

import concourse.bacc as bacc
import concourse.bass as bass
import concourse.mybir as mybir
import concourse.tile as tile
from concourse.bass_utils import run_bass_kernel_spmd

L = 45
START = 43
STOP = 44
LBAR = 46                  # labels + park row
PARK = 45
B = 1024
S = 512
NCORES = 8
BPC = B // NCORES          # 128 sequences per core
NG = 2                     # groups per core
WCOL = BPC // NG           # 64 columns per group
PR = NG * LBAR             # 92 partition rows for packed state
TS = S + 1                 # apps 0..512 (app 0 folded host-side, 512 appended absorb)
C = 8                      # time chunks
W = 64                     # apps per chunk window (C*W == 512 apps: 1..512)
WARM = 8                   # warmup apps for chunks 1..C-1
NSLOT = 2 * C - 1          # snapshot slots

DVE_CHUNKS = frozenset({0, 1, 2, 3})

# DMA split (in 64-col blocks) so each chunk starts compute early
PIECES0 = (9, 24, 32)      # chunk 0: 65 blocks (state0 + 64 apps)
PIECESW = (8, 24, 40)      # chunks 1..7: 72 blocks (8 warmup + 64 apps)

F32 = mybir.dt.float32
BF16 = mybir.dt.bfloat16


def _chunk_nblocks(c):
    return (1 + W) if c == 0 else (WARM + W)


def _build_nc():
    nc = bacc.Bacc("TRN2", target_bir_lowering=False, debug=False, num_devices=NCORES)
    e2t_dram = nc.dram_tensor("e2t", [PR, PR], BF16, kind="ExternalInput")
    g_dram = [
        nc.dram_tensor(f"g{c}", [PR, _chunk_nblocks(c) * WCOL], BF16,
                       kind="ExternalInput")
        for c in range(C)
    ]
    snaps_dram = nc.dram_tensor("snaps", [PR, NSLOT * WCOL], BF16,
                                kind="ExternalOutput")

    with tile.TileContext(nc) as tc:
        with (
            tc.tile_pool(name="const", bufs=1) as const_pool,
            tc.tile_pool(name="gtiles", bufs=1) as g_pool,
            tc.tile_pool(name="strip", bufs=1) as strip_pool,
        ):
            # Stage matmul lhsT through a DVE copy: matmult sem-wait encoding
            # is narrow, DMA completions fan out over many queue sems.
            e2t_st = const_pool.tile([PR, PR], BF16, tag="e2t_st")
            nc.sync.dma_start(e2t_st[:], e2t_dram[:])
            e2t = const_pool.tile([PR, PR], BF16, tag="e2t")
            nc.vector.tensor_copy(e2t[:], e2t_st[:])

            ones = const_pool.tile([PR, WCOL], BF16, tag="ones")
            nc.gpsimd.memset(ones[:], 1.0)

            # Per-chunk G pieces; issue piece 0 for every chunk first.
            gtiles = [[] for _ in range(C)]
            for p in range(3):
                for c in range(C):
                    pieces = PIECES0 if c == 0 else PIECESW
                    off = sum(pieces[:p])
                    nb = pieces[p]
                    gt = g_pool.tile([PR, nb * WCOL], F32 if False else BF16,
                                     tag=f"g{c}_{p}")
                    nc.sync.dma_start(
                        gt[:], g_dram[c][:, off * WCOL:(off + nb) * WCOL]
                    )
                    gtiles[c].append(gt)

            def gslice(c, blk):
                pieces = PIECES0 if c == 0 else PIECESW
                for p in range(3):
                    if blk < pieces[p]:
                        return gtiles[c][p][:, blk * WCOL:(blk + 1) * WCOL]
                    blk -= pieces[p]
                raise AssertionError

            snaps = strip_pool.tile([PR, NSLOT * WCOL], BF16, tag="snaps")

            def snap_slot(idx):
                return snaps[:, idx * WCOL:(idx + 1) * WCOL]

            with (
                tc.tile_pool(name="state", bufs=3) as state_pool,
                tc.tile_pool(name="tmp", bufs=2) as tmp_pool,
                tc.tile_pool(name="ps", bufs=1, space="PSUM") as ps_pool,
            ):
                # chunk 0 initial state: block 0 of g0, staged off the DMA tile
                st0 = state_pool.tile([PR, WCOL], BF16, tag="w0")
                nc.vector.tensor_copy(st0[:], gtiles[0][0][:, 0:WCOL])

                cur = [None] * C
                for c in range(C):
                    cur[c] = st0 if c == 0 else ones

                nsteps = [W if c == 0 else WARM + W for c in range(C)]
                for i in range(WARM + W):
                    for c in range(C):
                        if i >= nsteps[c]:
                            continue
                        ps = ps_pool.tile([PR, WCOL], F32, tag=f"s{c}")
                        nc.tensor.matmul(ps[:], e2t[:], cur[c][:],
                                         start=True, stop=True)
                        blk = (i + 1) if c == 0 else i
                        gsl = gslice(c, blk)
                        nw = state_pool.tile([PR, WCOL], BF16, tag=f"w{c}")
                        if c in DVE_CHUNKS:
                            nc.vector.tensor_mul(nw[:], gsl, ps[:])
                        else:
                            tp = tmp_pool.tile([PR, WCOL], F32, tag=f"t{c}")
                            nc.scalar.copy(tp[:], ps[:])
                            nc.gpsimd.tensor_mul(nw[:], gsl, tp[:])
                        cur[c] = nw
                        if c > 0 and i == WARM - 1:
                            slot = snap_slot(2 * c - 1)
                            if c in DVE_CHUNKS:
                                nc.vector.tensor_copy(slot, nw[:])
                            else:
                                nc.gpsimd.tensor_copy(slot, nw[:])
                        if i == nsteps[c] - 1:
                            slot = snap_slot(0 if c == 0 else 2 * c)
                            if c in DVE_CHUNKS:
                                nc.vector.tensor_copy(slot, nw[:])
                            else:
                                nc.gpsimd.tensor_copy(slot, nw[:])

            nc.sync.dma_start(snaps_dram[:], snaps[:])

    nc.compile()
    return nc


_NC_CACHE = {}


def _get_nc():
    if "nc" not in _NC_CACHE:
        _NC_CACHE["nc"] = _build_nc()
    return _NC_CACHE["nc"]


def _prep_inputs(logits, lens, transitions):
    """Host-side: exp/softmax prescale, park-row absorb rewrite, chunk packing."""
    logits = np.asarray(logits, np.float32)
    lens = np.asarray(lens, np.int64)
    T = np.asarray(transitions, np.float64)

    E = np.exp(T)
    Ebar = np.zeros((LBAR, LBAR), np.float64)
    Ebar[:L, :L] = E
    Ebar[PARK, :L] = E[STOP, :]
    Ebar[PARK, PARK] = 1.0

    e2t = np.zeros((PR, PR), np.float32)
    e2t[:LBAR, :LBAR] = Ebar.T
    e2t[LBAR:, LBAR:] = Ebar.T

    # softmax prescale + gamma drift compensation
    mx = logits.max(axis=2, keepdims=True)
    sumexp = np.exp(logits - mx).sum(axis=2)
    lse = mx[..., 0] + np.log(sumexp)                     # [B, S]
    sm = np.exp(logits - mx) / sumexp[..., None]          # [B, S, L]
    pbar = (Ebar[:L, :L] @ (np.ones(L) / L)).astype(np.float32)
    gamma = float(np.log(sm @ pbar).mean())

    active = np.arange(S)[None, :] < lens[:, None]        # [B, S]
    Gt = np.zeros((B, TS, LBAR), np.float32)
    Gt[:, :S, :L] = np.where(active[..., None], sm * np.float32(np.exp(-gamma)), 0.0)
    Gt[:, :S, PARK] = np.where(active, 0.0, 1.0)
    Gt[:, S, PARK] = 1.0

    corr = np.where(active, lse.astype(np.float64) + gamma, 0.0).sum(axis=1)

    state0 = Gt[:, 0, :] * Ebar[:, START].astype(np.float32)[None, :]  # [B, LBAR]

    e2t_b = e2t.astype(ml_dtypes.bfloat16)
    in_maps = []
    for cc in range(NCORES):
        sl = slice(cc * BPC, (cc + 1) * BPC)
        # [128, TS, 46] -> [2, 46, TS, 64] -> [92, TS, 64]
        arr = np.transpose(
            Gt[sl].reshape(NG, WCOL, TS, LBAR), (0, 3, 2, 1)
        ).reshape(PR, TS, WCOL)
        s0 = np.transpose(
            state0[sl].reshape(NG, WCOL, LBAR), (0, 2, 1)
        ).reshape(PR, 1, WCOL)
        m = {"e2t": e2t_b}
        for c in range(C):
            if c == 0:
                blocks = np.concatenate([s0, arr[:, 1:1 + W]], axis=1)
            else:
                t0 = 1 + W * c - WARM
                blocks = arr[:, t0:t0 + WARM + W]
            m[f"g{c}"] = np.ascontiguousarray(
                blocks.reshape(PR, _chunk_nblocks(c) * WCOL)
            ).astype(ml_dtypes.bfloat16)
        in_maps.append(m)
    return in_maps, corr, lens


def _postprocess(results, corr, lens):
    norm = np.empty(B, np.float64)
    for cc in range(NCORES):
        sn = np.asarray(results[cc]["snaps"]).astype(np.float64)
        sn = sn.reshape(PR, NSLOT, WCOL)
        for g in range(NG):
            rows = sn[g * LBAR:(g + 1) * LBAR]           # [46, NSLOT, 64]
            s = rows.sum(axis=0)                          # [NSLOT, 64]
            logz = np.log(s[0])
            for c in range(1, C - 1):
                logz += np.log(s[2 * c]) - np.log(s[2 * c - 1])
            park = rows[PARK, 2 * (C - 1)]                # final full state's park row
            logz += np.log(park) - np.log(s[2 * (C - 1) - 1])
            sl = slice(cc * BPC + g * WCOL, cc * BPC + (g + 1) * WCOL)
            norm[sl] = logz + corr[sl]
    return norm.astype(np.float32)


def kernel(logits, lens, transitions):
    nc = _get_nc()
    in_maps, corr, lens64 = _prep_inputs(logits, lens, transitions)
    res = run_bass_kernel_spmd(nc, in_maps, list(range(NCORES)))
    return _postprocess(res.results, corr, lens64)


# revision 6
# speedup vs baseline: 5.4748x; 1.1305x over previous
"""CRF forward (log partition) on 8 NeuronCores — chunked-parallel recurrence.

Math: the probability-space recurrence P_{t+1} = G_t o (E @ P_t) contracts
direction exponentially fast (products of positive matrices), so the 512
serial steps are split into C=12 time chunks run as independent streams,
each warm-started ~9 steps early from an all-ones state.  Host-side
stitching recovers log Z from per-chunk boundary row-sum ratios (the warmup
constant cancels); measured direction error after 8 steps is ~1e-5.

Range control without on-device renorm: active emission rows are prescaled
host-side by softmax times e^{-gamma}; the exact correction sum_t (LSE +
gamma) is added back on the host.  Absorbed steps (t >= len) park the
sequence's STOP projection in a dedicated 46th row per group whose
self-transition is exactly 1.0, so parked values are bit-stable in bf16.

Execution: 12 chunks form 2 lockstep cohorts of 6.  A cohort tick is ONE
PE matmul (lhsT = blockdiag(Ebar^T, Ebar^T) bf16, rhs = [92, 6*64] packed
states) and ONE DVE multiply (G-slice o PSUM -> next states), so the
PSUM-access cost and matmul fixed latency amortize over 6 chunks, and the
two cohorts keep PE and DVE pipelined against each other.
"""

import numpy as np
import ml_dtypes

import concourse.bacc as bacc
import concourse.bass as bass
import concourse.mybir as mybir
import concourse.tile as tile
from concourse.bass_utils import run_bass_kernel_spmd

L = 45
START = 43
STOP = 44
LBAR = 46                  # labels + park row
PARK = 45
B = 1024
S = 512
NCORES = 8
BPC = B // NCORES          # 128 sequences per core
NG = 2                     # groups per core
WCOL = BPC // NG           # 64 columns per group
PR = NG * LBAR             # 92 partition rows for packed state
TS = S + 1                 # apps 0..512 (app 0 folded host-side, 512 appended absorb)

C = 12                     # time chunks
NCOH = 2                   # lockstep cohorts
CPC = C // NCOH            # chunks per cohort
TICKS = 51                 # apps per chunk incl warmup
WARM = 9                   # warmup apps (chunks 1..C-1)
# windows: chunk 0 runs apps 1..51 exactly; chunks 1..10 cover 42 apps each,
# chunk 11 covers 41 apps + 1 pad absorb app (exact no-op on parked state).
W0 = TICKS
WC = 42
BOUNDS = [1, 1 + W0] + [1 + W0 + WC * c for c in range(1, C - 1)] + [TS]
assert BOUNDS[-2] + WC >= TS and len(BOUNDS) == C + 1

NSLOT = 2 * C - 1          # 11 start snaps + 12 end snaps
CW = CPC * WCOL            # cohort tile width (384)

F32 = mybir.dt.float32
BF16 = mybir.dt.bfloat16

# DMA pieces per cohort G tensor, in ticks
PIECES = (9, 16, 26)


def _build_nc():
    nc = bacc.Bacc("TRN2", target_bir_lowering=False, debug=False, num_devices=NCORES)
    e2t_dram = nc.dram_tensor("e2t", [PR, PR], BF16, kind="ExternalInput")
    s0_dram = nc.dram_tensor("s0", [PR, WCOL], BF16, kind="ExternalInput")
    g_dram = [
        nc.dram_tensor(f"g{k}", [PR, TICKS * CW], BF16, kind="ExternalInput")
        for k in range(NCOH)
    ]
    snaps_dram = nc.dram_tensor("snaps", [PR, NSLOT * WCOL], BF16,
                                kind="ExternalOutput")

    with tile.TileContext(nc) as tc:
        with (
            tc.tile_pool(name="const", bufs=1) as const_pool,
            tc.tile_pool(name="gtiles", bufs=1) as g_pool,
            tc.tile_pool(name="strip", bufs=1) as strip_pool,
            tc.tile_pool(name="state", bufs=2) as state_pool,
            tc.tile_pool(name="ps", bufs=2, space="PSUM") as ps_pool,
        ):
            # Stage matmul lhsT through a DVE copy: matmult sem-wait encoding
            # is narrow, DMA completions fan out over many queue sems.
            e2t_st = const_pool.tile([PR, PR], BF16, tag="e2t_st")
            nc.sync.dma_start(e2t_st[:], e2t_dram[:])
            e2t = const_pool.tile([PR, PR], BF16, tag="e2t")
            nc.vector.tensor_copy(e2t[:], e2t_st[:])
            s0_st = const_pool.tile([PR, WCOL], BF16, tag="s0_st")
            nc.sync.dma_start(s0_st[:], s0_dram[:])

            gtiles = [[] for _ in range(NCOH)]
            for p in range(3):
                for k in range(NCOH):
                    off = sum(PIECES[:p])
                    nb = PIECES[p]
                    gt = g_pool.tile([PR, nb * CW], BF16, tag=f"g{k}_{p}")
                    nc.sync.dma_start(gt[:], g_dram[k][:, off * CW:(off + nb) * CW])
                    gtiles[k].append(gt)

            def gslice(k, i):
                for p in range(3):
                    if i < PIECES[p]:
                        return gtiles[k][p][:, i * CW:(i + 1) * CW]
                    i -= PIECES[p]
                raise AssertionError

            snaps = strip_pool.tile([PR, NSLOT * WCOL], BF16, tag="snaps")

            # initial cohort states: ones everywhere; chunk 0 slice = state0
            cur = []
            for k in range(NCOH):
                st = state_pool.tile([PR, CW], BF16, tag=f"w{k}")
                nc.gpsimd.memset(st[:], 1.0)
                if k == 0:
                    nc.vector.tensor_copy(st[:, 0:WCOL], s0_st[:])
                cur.append(st)

            for i in range(TICKS):
                for k in range(NCOH):
                    ps = ps_pool.tile([PR, CW], F32, tag=f"s{k}")
                    nc.tensor.matmul(ps[:], e2t[:], cur[k][:],
                                     start=True, stop=True)
                    nw = state_pool.tile([PR, CW], BF16, tag=f"w{k}")
                    nc.vector.tensor_mul(nw[:], gslice(k, i), ps[:])
                    cur[k] = nw
                    if i == WARM - 1:
                        # start snapshots: chunks 1..11 (skip chunk 0) -> slots c-1
                        lo = 1 if k == 0 else 0
                        s0_slot = k * CPC + lo - 1
                        n_sl = CPC - lo
                        nc.gpsimd.tensor_copy(
                            snaps[:, s0_slot * WCOL:(s0_slot + n_sl) * WCOL],
                            nw[:, lo * WCOL:CPC * WCOL],
                        )
                    if i == TICKS - 1:
                        # end snapshots: all chunks, slots 11..22
                        base = (C - 1) + k * CPC
                        nc.gpsimd.tensor_copy(
                            snaps[:, base * WCOL:(base + CPC) * WCOL], nw[:]
                        )

            nc.sync.dma_start(snaps_dram[:], snaps[:])

    nc.compile()
    return nc


_NC_CACHE = {}


def _get_nc():
    if "nc" not in _NC_CACHE:
        _NC_CACHE["nc"] = _build_nc()
    return _NC_CACHE["nc"]


def _prep_inputs(logits, lens, transitions):
    """Host-side: exp/softmax prescale, park-row absorb rewrite, cohort packing."""
    logits = np.asarray(logits, np.float32)
    lens = np.asarray(lens, np.int64)
    T = np.asarray(transitions, np.float64)

    E = np.exp(T)
    Ebar = np.zeros((LBAR, LBAR), np.float64)
    Ebar[:L, :L] = E
    Ebar[PARK, :L] = E[STOP, :]
    Ebar[PARK, PARK] = 1.0

    e2t = np.zeros((PR, PR), np.float32)
    e2t[:LBAR, :LBAR] = Ebar.T
    e2t[LBAR:, LBAR:] = Ebar.T

    mx = logits.max(axis=2, keepdims=True)
    sumexp = np.exp(logits - mx).sum(axis=2)
    lse = mx[..., 0] + np.log(sumexp)                     # [B, S]
    sm = np.exp(logits - mx) / sumexp[..., None]          # [B, S, L]
    pbar = (Ebar[:L, :L] @ (np.ones(L) / L)).astype(np.float32)
    gamma = float(np.log(sm @ pbar).mean())

    active = np.arange(S)[None, :] < lens[:, None]        # [B, S]
    Gt = np.zeros((B, TS, LBAR), np.float32)
    Gt[:, :S, :L] = np.where(active[..., None], sm * np.float32(np.exp(-gamma)), 0.0)
    Gt[:, :S, PARK] = np.where(active, 0.0, 1.0)
    Gt[:, S, PARK] = 1.0

    corr = np.where(active, lse.astype(np.float64) + gamma, 0.0).sum(axis=1)

    state0 = Gt[:, 0, :] * Ebar[:, START].astype(np.float32)[None, :]  # [B, LBAR]

    # per-chunk app index at tick i (clamped to the pad absorb app TS-1... TS)
    app_idx = np.empty((C, TICKS), np.int64)
    for c in range(C):
        t0 = BOUNDS[c] - (0 if c == 0 else WARM)
        app_idx[c] = np.minimum(t0 + np.arange(TICKS), TS - 1)
        # chunk 11's final pad tick reuses the absorb app TS-1 (exact no-op)

    e2t_b = e2t.astype(ml_dtypes.bfloat16)
    in_maps = []
    for cc in range(NCORES):
        sl = slice(cc * BPC, (cc + 1) * BPC)
        # [128, TS, 46] -> [2, 46, TS, 64] -> [92, TS, 64]
        arr = np.transpose(
            Gt[sl].reshape(NG, WCOL, TS, LBAR), (0, 3, 2, 1)
        ).reshape(PR, TS, WCOL)
        s0 = np.ascontiguousarray(np.transpose(
            state0[sl].reshape(NG, WCOL, LBAR), (0, 2, 1)
        ).reshape(PR, WCOL)).astype(ml_dtypes.bfloat16)
        m = {"e2t": e2t_b, "s0": s0}
        for k in range(NCOH):
            # [92, TICKS, CPC, 64]: tick-major, chunk slices side by side
            chunks = app_idx[k * CPC:(k + 1) * CPC]       # [CPC, TICKS]
            blocks = arr[:, chunks.T]                     # [92, TICKS, CPC, 64]
            m[f"g{k}"] = np.ascontiguousarray(
                blocks.reshape(PR, TICKS * CW)
            ).astype(ml_dtypes.bfloat16)
        in_maps.append(m)
    return in_maps, corr, lens


def _postprocess(results, corr, lens):
    norm = np.empty(B, np.float64)
    for cc in range(NCORES):
        sn = np.asarray(results[cc]["snaps"]).astype(np.float64)
        sn = sn.reshape(PR, NSLOT, WCOL)
        for g in range(NG):
            rows = sn[g * LBAR:(g + 1) * LBAR]           # [46, NSLOT, 64]
            s = rows.sum(axis=0)                          # [NSLOT, 64]
            # slots: 0..10 = start snaps of chunks 1..11; 11..22 = end snaps
            logz = np.log(s[C - 1])                       # chunk 0 end
            for c in range(1, C - 1):
                logz += np.log(s[C - 1 + c]) - np.log(s[c - 1])
            park = rows[PARK, NSLOT - 1]                  # final state's park row
            logz += np.log(park) - np.log(s[C - 2])
            sl = slice(cc * BPC + g * WCOL, cc * BPC + (g + 1) * WCOL)
            norm[sl] = logz + corr[sl]
    return norm.astype(np.float32)


def kernel(logits, lens, transitions):
    nc = _get_nc()
    in_maps, corr, lens64 = _prep_inputs(logits, lens, transitions)
    res = run_bass_kernel_spmd(nc, in_maps, list(range(NCORES)))
    return _postprocess(res.results, corr, lens64)


# revision 7
# speedup vs baseline: 5.8894x; 1.0757x over previous
"""CRF forward (log partition) on 8 NeuronCores — chunked-parallel recurrence.

Math: the probability-space recurrence P_{t+1} = G_t o (E @ P_t) contracts
direction exponentially fast (products of positive matrices), so the 512
serial steps are split into C=12 time chunks run as independent streams,
each warm-started ~9 steps early from an all-ones state.  Host-side
stitching recovers log Z from per-chunk boundary row-sum ratios (the warmup
constant cancels); measured direction error after 8 steps is ~1e-5.

Range control without on-device renorm: active emission rows are prescaled
host-side by softmax times e^{-gamma}; the exact correction sum_t (LSE +
gamma) is added back on the host.  Absorbed steps (t >= len) park the
sequence's STOP projection in a dedicated 46th row per group whose
self-transition is exactly 1.0, so parked values are bit-stable in bf16.

Execution: 12 chunks form 2 lockstep cohorts of 6.  A cohort tick is ONE
PE matmul (lhsT = blockdiag(Ebar^T, Ebar^T) bf16, rhs = [92, 6*64] packed
states) and ONE DVE multiply (G-slice o PSUM -> next states), so the
PSUM-access cost and matmul fixed latency amortize over 6 chunks, and the
two cohorts keep PE and DVE pipelined against each other.
"""

import numpy as np
import ml_dtypes

import concourse.bacc as bacc
import concourse.bass as bass
import concourse.mybir as mybir
import concourse.tile as tile
from concourse.bass_utils import run_bass_kernel_spmd

L = 45
START = 43
STOP = 44
LBAR = 46                  # labels + park row
PARK = 45
B = 1024
S = 512
NCORES = 8
BPC = B // NCORES          # 128 sequences per core
NG = 2                     # groups per core
WCOL = BPC // NG           # 64 columns per group
PR = NG * LBAR             # 92 partition rows for packed state
TS = S + 1                 # apps 0..512 (app 0 folded host-side, 512 appended absorb)

C = 12                     # time chunks
NCOH = 2                   # lockstep cohorts
CPC = C // NCOH            # chunks per cohort
TICKS = 51                 # apps per chunk incl warmup
WARM = 9                   # warmup apps (chunks 1..C-1)
# windows: chunk 0 runs apps 1..51 exactly; chunks 1..10 cover 42 apps each,
# chunk 11 covers 41 apps + 1 pad absorb app (exact no-op on parked state).
W0 = TICKS
WC = 42
BOUNDS = [1, 1 + W0] + [1 + W0 + WC * c for c in range(1, C - 1)] + [TS]
assert BOUNDS[-2] + WC >= TS and len(BOUNDS) == C + 1

NSLOT = 2 * C - 1          # 11 start snaps + 12 end snaps
CW = CPC * WCOL            # cohort tile width (384)

F32 = mybir.dt.float32
BF16 = mybir.dt.bfloat16

# DMA pieces per cohort G tensor, in ticks
PIECES = (3, 6, 10, 14, 18)


def _build_nc():
    nc = bacc.Bacc("TRN2", target_bir_lowering=False, debug=False, num_devices=NCORES)
    e2t_dram = nc.dram_tensor("e2t", [PR, PR], BF16, kind="ExternalInput")
    s0_dram = nc.dram_tensor("s0", [PR, WCOL], BF16, kind="ExternalInput")
    g_dram = [
        nc.dram_tensor(f"g{k}", [PR, TICKS * CW], BF16, kind="ExternalInput")
        for k in range(NCOH)
    ]
    snaps_dram = nc.dram_tensor("snaps", [PR, NSLOT * WCOL], BF16,
                                kind="ExternalOutput")

    with tile.TileContext(nc) as tc:
        with (
            tc.tile_pool(name="const", bufs=1) as const_pool,
            tc.tile_pool(name="gtiles", bufs=1) as g_pool,
            tc.tile_pool(name="strip", bufs=1) as strip_pool,
            tc.tile_pool(name="state", bufs=3) as state_pool,
            tc.tile_pool(name="ps", bufs=2, space="PSUM") as ps_pool,
        ):
            # Stage matmul lhsT through a DVE copy: matmult sem-wait encoding
            # is narrow, DMA completions fan out over many queue sems.
            e2t_st = const_pool.tile([PR, PR], BF16, tag="e2t_st")
            nc.sync.dma_start(e2t_st[:], e2t_dram[:])
            e2t = const_pool.tile([PR, PR], BF16, tag="e2t")
            nc.vector.tensor_copy(e2t[:], e2t_st[:])
            s0_st = const_pool.tile([PR, WCOL], BF16, tag="s0_st")
            nc.sync.dma_start(s0_st[:], s0_dram[:])

            gtiles = [[] for _ in range(NCOH)]
            for p in range(len(PIECES)):
                for k in range(NCOH):
                    off = sum(PIECES[:p])
                    nb = PIECES[p]
                    gt = g_pool.tile([PR, nb * CW], BF16, tag=f"g{k}_{p}")
                    nc.sync.dma_start(gt[:], g_dram[k][:, off * CW:(off + nb) * CW])
                    gtiles[k].append(gt)

            def gslice(k, i):
                for p in range(len(PIECES)):
                    if i < PIECES[p]:
                        return gtiles[k][p][:, i * CW:(i + 1) * CW]
                    i -= PIECES[p]
                raise AssertionError

            snaps = strip_pool.tile([PR, NSLOT * WCOL], BF16, tag="snaps")

            # initial cohort states: ones everywhere; chunk 0 slice = state0
            cur = []
            for k in range(NCOH):
                st = state_pool.tile([PR, CW], BF16, tag=f"w{k}")
                nc.gpsimd.memset(st[:], 1.0)
                if k == 0:
                    nc.vector.tensor_copy(st[:, 0:WCOL], s0_st[:])
                cur.append(st)

            for i in range(TICKS):
                for k in range(NCOH):
                    ps = ps_pool.tile([PR, CW], F32, tag=f"s{k}")
                    nc.tensor.matmul(ps[:], e2t[:], cur[k][:],
                                     start=True, stop=True)
                    nw = state_pool.tile([PR, CW], BF16, tag=f"w{k}")
                    nc.vector.tensor_mul(nw[:], gslice(k, i), ps[:])
                    cur[k] = nw
                    if i == WARM - 1:
                        # start snapshots: chunks 1..11 (skip chunk 0) -> slots c-1
                        lo = 1 if k == 0 else 0
                        s0_slot = k * CPC + lo - 1
                        n_sl = CPC - lo
                        nc.scalar.copy(
                            snaps[:, s0_slot * WCOL:(s0_slot + n_sl) * WCOL],
                            nw[:, lo * WCOL:CPC * WCOL],
                        )
                    if i == TICKS - 1:
                        # end snapshots: all chunks, slots 11..22
                        base = (C - 1) + k * CPC
                        nc.scalar.copy(
                            snaps[:, base * WCOL:(base + CPC) * WCOL], nw[:]
                        )

            nc.sync.dma_start(snaps_dram[:], snaps[:])

    nc.compile()
    return nc


_NC_CACHE = {}


def _get_nc():
    if "nc" not in _NC_CACHE:
        _NC_CACHE["nc"] = _build_nc()
    return _NC_CACHE["nc"]


def _prep_inputs(logits, lens, transitions):
    """Host-side: exp/softmax prescale, park-row absorb rewrite, cohort packing."""
    logits = np.asarray(logits, np.float32)
    lens = np.asarray(lens, np.int64)
    T = np.asarray(transitions, np.float64)

    E = np.exp(T)
    Ebar = np.zeros((LBAR, LBAR), np.float64)
    Ebar[:L, :L] = E
    Ebar[PARK, :L] = E[STOP, :]
    Ebar[PARK, PARK] = 1.0

    e2t = np.zeros((PR, PR), np.float32)
    e2t[:LBAR, :LBAR] = Ebar.T
    e2t[LBAR:, LBAR:] = Ebar.T

    mx = logits.max(axis=2, keepdims=True)
    sumexp = np.exp(logits - mx).sum(axis=2)
    lse = mx[..., 0] + np.log(sumexp)                     # [B, S]
    sm = np.exp(logits - mx) / sumexp[..., None]          # [B, S, L]
    pbar = (Ebar[:L, :L] @ (np.ones(L) / L)).astype(np.float32)
    gamma = float(np.log(sm @ pbar).mean())

    active = np.arange(S)[None, :] < lens[:, None]        # [B, S]
    Gt = np.zeros((B, TS, LBAR), np.float32)
    Gt[:, :S, :L] = np.where(active[..., None], sm * np.float32(np.exp(-gamma)), 0.0)
    Gt[:, :S, PARK] = np.where(active, 0.0, 1.0)
    Gt[:, S, PARK] = 1.0

    corr = np.where(active, lse.astype(np.float64) + gamma, 0.0).sum(axis=1)

    state0 = Gt[:, 0, :] * Ebar[:, START].astype(np.float32)[None, :]  # [B, LBAR]

    # per-chunk app index at tick i (clamped to the pad absorb app TS-1... TS)
    app_idx = np.empty((C, TICKS), np.int64)
    for c in range(C):
        t0 = BOUNDS[c] - (0 if c == 0 else WARM)
        app_idx[c] = np.minimum(t0 + np.arange(TICKS), TS - 1)
        # chunk 11's final pad tick reuses the absorb app TS-1 (exact no-op)

    e2t_b = e2t.astype(ml_dtypes.bfloat16)
    in_maps = []
    for cc in range(NCORES):
        sl = slice(cc * BPC, (cc + 1) * BPC)
        # [128, TS, 46] -> [2, 46, TS, 64] -> [92, TS, 64]
        arr = np.transpose(
            Gt[sl].reshape(NG, WCOL, TS, LBAR), (0, 3, 2, 1)
        ).reshape(PR, TS, WCOL)
        s0 = np.ascontiguousarray(np.transpose(
            state0[sl].reshape(NG, WCOL, LBAR), (0, 2, 1)
        ).reshape(PR, WCOL)).astype(ml_dtypes.bfloat16)
        m = {"e2t": e2t_b, "s0": s0}
        for k in range(NCOH):
            # [92, TICKS, CPC, 64]: tick-major, chunk slices side by side
            chunks = app_idx[k * CPC:(k + 1) * CPC]       # [CPC, TICKS]
            blocks = arr[:, chunks.T]                     # [92, TICKS, CPC, 64]
            m[f"g{k}"] = np.ascontiguousarray(
                blocks.reshape(PR, TICKS * CW)
            ).astype(ml_dtypes.bfloat16)
        in_maps.append(m)
    return in_maps, corr, lens


def _postprocess(results, corr, lens):
    norm = np.empty(B, np.float64)
    for cc in range(NCORES):
        sn = np.asarray(results[cc]["snaps"]).astype(np.float64)
        sn = sn.reshape(PR, NSLOT, WCOL)
        for g in range(NG):
            rows = sn[g * LBAR:(g + 1) * LBAR]           # [46, NSLOT, 64]
            s = rows.sum(axis=0)                          # [NSLOT, 64]
            # slots: 0..10 = start snaps of chunks 1..11; 11..22 = end snaps
            logz = np.log(s[C - 1])                       # chunk 0 end
            for c in range(1, C - 1):
                logz += np.log(s[C - 1 + c]) - np.log(s[c - 1])
            park = rows[PARK, NSLOT - 1]                  # final state's park row
            logz += np.log(park) - np.log(s[C - 2])
            sl = slice(cc * BPC + g * WCOL, cc * BPC + (g + 1) * WCOL)
            norm[sl] = logz + corr[sl]
    return norm.astype(np.float32)


def kernel(logits, lens, transitions):
    nc = _get_nc()
    in_maps, corr, lens64 = _prep_inputs(logits, lens, transitions)
    res = run_bass_kernel_spmd(nc, in_maps, list(range(NCORES)))
    return _postprocess(res.results, corr, lens64)


# revision 13
# speedup vs baseline: 5.8944x; 1.0009x over previous
"""CRF forward (log partition) on 8 NeuronCores — chunked-parallel recurrence.

Math: the probability-space recurrence P_{t+1} = G_t o (E @ P_t) contracts
direction exponentially fast (products of positive matrices), so the 512
serial steps are split into C=12 time chunks run as independent streams,
each warm-started ~9 steps early from an all-ones state.  Host-side
stitching recovers log Z from per-chunk boundary row-sum ratios (the warmup
constant cancels); measured direction error after 8 steps is ~1e-5.

Range control without on-device renorm: active emission rows are prescaled
host-side by softmax times e^{-gamma}; the exact correction sum_t (LSE +
gamma) is added back on the host.  Absorbed steps (t >= len) park the
sequence's STOP projection in a dedicated 46th row per group whose
self-transition is exactly 1.0, so parked values are bit-stable in bf16.

Execution: 12 chunks form 2 lockstep cohorts of 6.  A cohort tick is ONE
PE matmul (lhsT = blockdiag(Ebar^T, Ebar^T) bf16, rhs = [92, 6*64] packed
states) and ONE DVE multiply (G-slice o PSUM -> next states), so the
PSUM-access cost and matmul fixed latency amortize over 6 chunks, and the
two cohorts keep PE and DVE pipelined against each other.
"""

import numpy as np
import ml_dtypes

import concourse.bacc as bacc
import concourse.bass as bass
import concourse.mybir as mybir
import concourse.tile as tile
from concourse.bass_utils import run_bass_kernel_spmd

L = 45
START = 43
STOP = 44
LBAR = 46                  # labels + park row
PARK = 45
B = 1024
S = 512
NCORES = 8
BPC = B // NCORES          # 128 sequences per core
NG = 2                     # groups per core
WCOL = BPC // NG           # 64 columns per group
PR = NG * LBAR             # 92 partition rows for packed state
TS = S + 1                 # apps 0..512 (app 0 folded host-side, 512 appended absorb)

C = 12                     # time chunks
NCOH = 2                   # lockstep cohorts
CPC = C // NCOH            # chunks per cohort
TICKS = 51                 # apps per chunk incl warmup
WARM = 9                   # warmup apps (chunks 1..C-1)
# windows: chunk 0 runs apps 1..51 exactly; chunks 1..10 cover 42 apps each,
# chunk 11 covers 41 apps + 1 pad absorb app (exact no-op on parked state).
W0 = TICKS
WC = 42
BOUNDS = [1, 1 + W0] + [1 + W0 + WC * c for c in range(1, C - 1)] + [TS]
assert BOUNDS[-2] + WC >= TS and len(BOUNDS) == C + 1

NSLOT = 2 * C - 1          # 11 start snaps + 12 end snaps
CW = CPC * WCOL            # cohort tile width (384)

F32 = mybir.dt.float32
BF16 = mybir.dt.bfloat16

# DMA pieces per cohort G tensor, in ticks
PIECES = (3, 6, 10, 14, 18)


def _build_nc():
    nc = bacc.Bacc("TRN2", target_bir_lowering=False, debug=False, num_devices=NCORES)
    e2t_dram = nc.dram_tensor("e2t", [PR, PR], BF16, kind="ExternalInput")
    s0_dram = nc.dram_tensor("s0", [PR, WCOL], BF16, kind="ExternalInput")
    g_dram = [
        nc.dram_tensor(f"g{k}", [PR, TICKS * CW], BF16, kind="ExternalInput")
        for k in range(NCOH)
    ]
    snaps_dram = nc.dram_tensor("snaps", [PR, NSLOT * WCOL], BF16,
                                kind="ExternalOutput")

    with tile.TileContext(nc) as tc:
        with (
            tc.tile_pool(name="const", bufs=1) as const_pool,
            tc.tile_pool(name="gtiles", bufs=1) as g_pool,
            tc.tile_pool(name="strip", bufs=1) as strip_pool,
            tc.tile_pool(name="state", bufs=3) as state_pool,
            tc.tile_pool(name="ps", bufs=2, space="PSUM") as ps_pool,
        ):
            # Stage matmul lhsT through a DVE copy: matmult sem-wait encoding
            # is narrow, DMA completions fan out over many queue sems.
            e2t_st = const_pool.tile([PR, PR], BF16, tag="e2t_st")
            nc.sync.dma_start(e2t_st[:], e2t_dram[:])
            e2t = const_pool.tile([PR, PR], BF16, tag="e2t")
            nc.vector.tensor_copy(e2t[:], e2t_st[:])
            s0_st = const_pool.tile([PR, WCOL], BF16, tag="s0_st")
            nc.sync.dma_start(s0_st[:], s0_dram[:])

            # Spread G DMAs over four engine queues: a single queue stripes
            # over only 4 of the 16 DMA engines (~75 GB/s); four queues reach
            # the full fabric.
            dma_engines = [nc.sync, nc.scalar, nc.gpsimd]
            gtiles = [[] for _ in range(NCOH)]
            for p in range(len(PIECES)):
                for k in range(NCOH):
                    off = sum(PIECES[:p])
                    nb = PIECES[p]
                    gt = g_pool.tile([PR, nb * CW], BF16, tag=f"g{k}_{p}")
                    eng = dma_engines[(2 * p + k) % 3]
                    eng.dma_start(gt[:], g_dram[k][:, off * CW:(off + nb) * CW])
                    gtiles[k].append(gt)

            def gslice(k, i):
                for p in range(len(PIECES)):
                    if i < PIECES[p]:
                        return gtiles[k][p][:, i * CW:(i + 1) * CW]
                    i -= PIECES[p]
                raise AssertionError

            snaps = strip_pool.tile([PR, NSLOT * WCOL], BF16, tag="snaps")

            # initial cohort states: ones everywhere; chunk 0 slice = state0
            cur = []
            for k in range(NCOH):
                st = state_pool.tile([PR, CW], BF16, tag=f"w{k}")
                nc.gpsimd.memset(st[:], 1.0)
                if k == 0:
                    nc.vector.tensor_copy(st[:, 0:WCOL], s0_st[:])
                cur.append(st)

            for i in range(TICKS):
                for k in range(NCOH):
                    ps = ps_pool.tile([PR, CW], F32, tag=f"s{k}")
                    nc.tensor.matmul(ps[:], e2t[:], cur[k][:],
                                     start=True, stop=True)
                    nw = state_pool.tile([PR, CW], BF16, tag=f"w{k}")
                    nc.vector.tensor_mul(nw[:], gslice(k, i), ps[:])
                    cur[k] = nw
                    if i == WARM - 1:
                        # start snapshots: chunks 1..11 (skip chunk 0) -> slots c-1
                        lo = 1 if k == 0 else 0
                        s0_slot = k * CPC + lo - 1
                        n_sl = CPC - lo
                        nc.scalar.copy(
                            snaps[:, s0_slot * WCOL:(s0_slot + n_sl) * WCOL],
                            nw[:, lo * WCOL:CPC * WCOL],
                        )
                        if k == NCOH - 1:
                            # ship start snaps now; end slots go at the end
                            nc.scalar.dma_start(
                                snaps_dram[:, 0:(C - 1) * WCOL],
                                snaps[:, 0:(C - 1) * WCOL],
                            )
                    if i == TICKS - 1:
                        # end snapshots: all chunks, slots 11..22
                        base = (C - 1) + k * CPC
                        nc.scalar.copy(
                            snaps[:, base * WCOL:(base + CPC) * WCOL], nw[:]
                        )

            nc.sync.dma_start(
                snaps_dram[:, (C - 1) * WCOL:], snaps[:, (C - 1) * WCOL:]
            )

    nc.compile()
    return nc


_NC_CACHE = {}


def _get_nc():
    if "nc" not in _NC_CACHE:
        _NC_CACHE["nc"] = _build_nc()
    return _NC_CACHE["nc"]


def _prep_inputs(logits, lens, transitions):
    """Host-side: exp/softmax prescale, park-row absorb rewrite, cohort packing."""
    logits = np.asarray(logits, np.float32)
    lens = np.asarray(lens, np.int64)
    T = np.asarray(transitions, np.float64)

    E = np.exp(T)
    Ebar = np.zeros((LBAR, LBAR), np.float64)
    Ebar[:L, :L] = E
    Ebar[PARK, :L] = E[STOP, :]
    Ebar[PARK, PARK] = 1.0

    e2t = np.zeros((PR, PR), np.float32)
    e2t[:LBAR, :LBAR] = Ebar.T
    e2t[LBAR:, LBAR:] = Ebar.T

    mx = logits.max(axis=2, keepdims=True)
    sumexp = np.exp(logits - mx).sum(axis=2)
    lse = mx[..., 0] + np.log(sumexp)                     # [B, S]
    sm = np.exp(logits - mx) / sumexp[..., None]          # [B, S, L]
    pbar = (Ebar[:L, :L] @ (np.ones(L) / L)).astype(np.float32)
    gamma = float(np.log(sm @ pbar).mean())

    active = np.arange(S)[None, :] < lens[:, None]        # [B, S]
    Gt = np.zeros((B, TS, LBAR), np.float32)
    Gt[:, :S, :L] = np.where(active[..., None], sm * np.float32(np.exp(-gamma)), 0.0)
    Gt[:, :S, PARK] = np.where(active, 0.0, 1.0)
    Gt[:, S, PARK] = 1.0

    corr = np.where(active, lse.astype(np.float64) + gamma, 0.0).sum(axis=1)

    state0 = Gt[:, 0, :] * Ebar[:, START].astype(np.float32)[None, :]  # [B, LBAR]

    # per-chunk app index at tick i (clamped to the pad absorb app TS-1... TS)
    app_idx = np.empty((C, TICKS), np.int64)
    for c in range(C):
        t0 = BOUNDS[c] - (0 if c == 0 else WARM)
        app_idx[c] = np.minimum(t0 + np.arange(TICKS), TS - 1)
        # chunk 11's final pad tick reuses the absorb app TS-1 (exact no-op)

    e2t_b = e2t.astype(ml_dtypes.bfloat16)
    in_maps = []
    for cc in range(NCORES):
        sl = slice(cc * BPC, (cc + 1) * BPC)
        # [128, TS, 46] -> [2, 46, TS, 64] -> [92, TS, 64]
        arr = np.transpose(
            Gt[sl].reshape(NG, WCOL, TS, LBAR), (0, 3, 2, 1)
        ).reshape(PR, TS, WCOL)
        s0 = np.ascontiguousarray(np.transpose(
            state0[sl].reshape(NG, WCOL, LBAR), (0, 2, 1)
        ).reshape(PR, WCOL)).astype(ml_dtypes.bfloat16)
        m = {"e2t": e2t_b, "s0": s0}
        for k in range(NCOH):
            # [92, TICKS, CPC, 64]: tick-major, chunk slices side by side
            chunks = app_idx[k * CPC:(k + 1) * CPC]       # [CPC, TICKS]
            blocks = arr[:, chunks.T]                     # [92, TICKS, CPC, 64]
            m[f"g{k}"] = np.ascontiguousarray(
                blocks.reshape(PR, TICKS * CW)
            ).astype(ml_dtypes.bfloat16)
        in_maps.append(m)
    return in_maps, corr, lens


def _postprocess(results, corr, lens):
    norm = np.empty(B, np.float64)
    for cc in range(NCORES):
        sn = np.asarray(results[cc]["snaps"]).astype(np.float64)
        sn = sn.reshape(PR, NSLOT, WCOL)
        for g in range(NG):
            rows = sn[g * LBAR:(g + 1) * LBAR]           # [46, NSLOT, 64]
            s = rows.sum(axis=0)                          # [NSLOT, 64]
            # slots: 0..10 = start snaps of chunks 1..11; 11..22 = end snaps
            logz = np.log(s[C - 1])                       # chunk 0 end
            for c in range(1, C - 1):
                logz += np.log(s[C - 1 + c]) - np.log(s[c - 1])
            park = rows[PARK, NSLOT - 1]                  # final state's park row
            logz += np.log(park) - np.log(s[C - 2])
            sl = slice(cc * BPC + g * WCOL, cc * BPC + (g + 1) * WCOL)
            norm[sl] = logz + corr[sl]
    return norm.astype(np.float32)


def kernel(logits, lens, transitions):
    nc = _get_nc()
    in_maps, corr, lens64 = _prep_inputs(logits, lens, transitions)
    res = run_bass_kernel_spmd(nc, in_maps, list(range(NCORES)))
    return _postprocess(res.results, corr, lens64)
